# revision 3
# baseline (speedup 1.0000x reference)
"""Trainium2 Bass kernel: per-channel cubic B-spline activation (KAN-style).

y[..., c] = sum_k W[c, k] * B_k(x[..., c])   with cubic B-spline bases B_k on a
uniform 12-point grid.

Math: for each channel c the map x -> y is a piecewise cubic, zero outside
[t0, t11], C^2 everywhere.  Such a function is exactly

    y_c(x) = sum_{m=0}^{10} g_m(c) * relu(min(x, t11) - t_m)^3

(truncated-power / hinge representation; the clamp at t11 makes the value 0
beyond the support since y_c(t11) = 0).

Layout: the host pre-transposes each core's shard to channel-on-partition
form [128, S] with partition p holding channel p % 32, so every hinge is ONE
contiguous fused DVE op over a full [128, F] tile: the per-channel weight
rho = cbrt(g) rides the per-partition scalar slot (s0 as a [128,1] AP), and
t_m / (t11 - t_m) are global immediates.  11 streaming passes per tile total.

Sharding: pure data parallel over the batch axis, 2 batches per core x 8 cores.
"""

import sys

sys.path.insert(0, "/opt/trn_rl_repo")

import numpy as np

# ---- hardcoded problem geometry ----
B, H, WIDTH, C = 16, 256, 256, 32
N_CORES = 8
PIX = (B // N_CORES) * H * WIDTH  # 131072 pixels per core
P = 128  # SBUF partitions
J = P // C  # 4 pixel-groups stacked per partition block
S = PIX // J  # 32768 free elements per partition
F = 8192  # free elements per tile
T = S // F  # 4 tiles per core
N_KNOTS = 12
N_HINGE = 11  # knots t0..t10 carry hinges; t11 handled by clamp

_STATE: dict = {}


# --------------------------------------------------------------------------
# numpy reference pieces (fp64) for coefficient extraction + self check
# --------------------------------------------------------------------------
def _bases_np(x, grid, order=3):
    xg = x[..., None]
    bases = ((xg >= grid[:-1]) & (xg < grid[1:])).astype(np.float64)
    for k in range(1, order + 1):
        left = (xg - grid[: -(k + 1)]) / (grid[k:-1] - grid[: -(k + 1)]) * bases[..., :-1]
        right = (grid[k + 1 :] - xg) / (grid[k + 1 :] - grid[1:-k]) * bases[..., 1:]
        bases = left + right
    return bases  # [..., len(grid)-1-order]


def _hinge_coeffs(grid, W):
    """g[c, m], m=0..10 with y_c(x) = sum_m g[c,m] (x - t_m)_+^3 on [t0, t11]."""
    g64 = grid.astype(np.float64)
    W64 = W.astype(np.float64)
    n_iv = 11  # intervals inside the support [t0, t11]
    a3 = np.zeros((C, n_iv))
    for i in range(n_iv):
        xs = np.linspace(g64[i], g64[i + 1], 6)[1:-1]  # 4 interior points
        bas = _bases_np(xs, g64)  # [4, 8]
        ys = bas @ W64.T  # [4, C]
        for c in range(C):
            a3[c, i] = np.polyfit(xs, ys[:, c], 3)[0]
    g = np.diff(np.concatenate([np.zeros((C, 1)), a3], axis=1), axis=1)  # [C, 11]
    return g


def _check_hinges(grid, W, g):
    """fp64 sanity check of the hinge model against the Cox-de Boor reference."""
    rng = np.random.default_rng(0)
    xs = rng.uniform(grid[0] - 0.5, grid[-1] + 0.5, 20000)
    ref = _bases_np(xs, grid.astype(np.float64)) @ W.astype(np.float64).T  # [n, C]
    xc = np.minimum(xs, np.float64(grid[-1]))
    hin = np.maximum(xc[:, None] - grid.astype(np.float64)[None, :N_HINGE], 0.0) ** 3
    mdl = hin @ g.T  # [n, C]
    err = np.abs(mdl - ref).max()
    scale = max(np.abs(ref).max(), 1e-30)
    assert err <= 1e-6 * scale + 1e-9, f"hinge model mismatch: {err=} {scale=}"


# --------------------------------------------------------------------------
# custom DVE ops
# --------------------------------------------------------------------------
def _register_dve_ops():
    if "ops" in _STATE:
        return _STATE["ops"]
    from concourse.dve_ops import (
        CUSTOM_DVE_SPECS,
        OPS,
        DveOp,
        _SUB_OPCODE_FOR_NAME,
    )
    from concourse.dve_spec import C0, C1, C2, Spec, Src0, Src1, _has_src1, lower, minn, relu, sq
    from concourse.dve_uop import DveOpSpec

    def make(name, spec):
        if name in _SUB_OPCODE_FOR_NAME:  # already registered in this process
            return next(op for op in OPS if op.name == name)
        opcode = max(_SUB_OPCODE_FOR_NAME.values()) + 1
        assert opcode < 0x20
        shas = {}
        for ver in ("v3", "v4"):
            s = DveOpSpec(
                name=name, opcode=opcode, uops=lower(spec, ver=ver), rd1_en=_has_src1(spec)
            )
            shas[ver] = s.sha(ver)
        op = DveOp(name, spec, subdim=False, uops_sha=shas)
        OPS.append(op)
        _SUB_OPCODE_FOR_NAME[name] = opcode
        CUSTOM_DVE_SPECS[name] = spec
        return op

    # Constant binding: C2 = imm2 -> tau_m (global), C1 = s1 -> hi_m (global),
    # C0 = s0 -> rho[p, m] (per-partition AP).
    # w = relu(min(x - tau, hi)) * rho ; term = w^3  (rho = cbrt(g), sign kept)
    _w = relu(minn(Src0 - C2, C1)) * C0
    init = make(
        "BSACT_HINGE_INIT_P",
        Spec(
            body=sq(_w) * _w,
            reference=lambda in0, in1, s0, s1, imm2: (
                np.maximum(np.minimum(in0.astype(np.float32) - imm2, s1), 0) * s0
            )
            ** 3,
        ),
    )
    _w2 = relu(minn(Src0 - C2, C1)) * C0
    acc = make(
        "BSACT_HINGE_ACC_P",
        Spec(
            body=Src1 + sq(_w2) * _w2,
            reference=lambda in0, in1, s0, s1, imm2: in1
            + (np.maximum(np.minimum(in0.astype(np.float32) - imm2, s1), 0) * s0) ** 3,
        ),
    )
    _STATE["ops"] = (init, acc)
    return init, acc


# --------------------------------------------------------------------------
# bass module
# --------------------------------------------------------------------------
def build_module(taus, his, rho128, n_tiles=T, free=F, reps=1, n_hinges=N_HINGE, dma_only=False):
    """taus[m], his[m]: knot / clamp constants; rho128[p, m] = cbrt(g)[p%32, m]."""
    import concourse.bacc as bacc
    import concourse.tile as tile
    from concourse import mybir

    op_init, op_acc = _register_dve_ops()

    nc = bacc.Bacc("TRN2", target_bir_lowering=False, debug=False, num_devices=N_CORES)
    x_d = nc.dram_tensor("x0", [n_tiles, P, free], mybir.dt.float32, kind="ExternalInput").ap()
    rho_d = nc.dram_tensor("rho0", [P, N_HINGE], mybir.dt.float32, kind="ExternalInput").ap()
    y_d = nc.dram_tensor("y0", [n_tiles, P, free], mybir.dt.float32, kind="ExternalOutput").ap()

    with tile.TileContext(nc) as tc:
        with (
            tc.tile_pool(name="const", bufs=1) as cp,
            tc.tile_pool(name="xin", bufs=3) as xp,
            tc.tile_pool(name="acc", bufs=2) as ac,
        ):
            rt = cp.tile([P, N_HINGE], mybir.dt.float32)
            nc.sync.dma_start(rt[:], rho_d)
            rho_ap = rt[:]
            for _ in range(reps):
                for t in range(n_tiles):
                    xt = xp.tile([P, free], mybir.dt.float32)
                    nc.sync.dma_start(xt[:], x_d[t])
                    if dma_only:
                        nc.sync.dma_start(y_d[t], xt[:])
                        continue
                    at = ac.tile([P, free], mybir.dt.float32)
                    xv = xt[:]
                    av = at[:]
                    nc.vector._custom_dve(
                        op_init, out=av, in0=xv,
                        s0=rho_ap[:, 0:1], s1=float(his[0]), imm2=float(taus[0]),
                    )
                    for m in range(1, n_hinges):
                        nc.vector._custom_dve(
                            op_acc, out=av, in0=xv, in1=av,
                            s0=rho_ap[:, m : m + 1], s1=float(his[m]), imm2=float(taus[m]),
                        )
                    nc.sync.dma_start(y_d[t], at[:])
    nc.compile()
    return nc


def _constants(grid, W):
    g = _hinge_coeffs(grid, W)  # [C, 11] fp64
    _check_hinges(grid, W, g)
    rhos = np.cbrt(g)  # sign-preserving cube root
    taus = grid[:N_HINGE].astype(np.float64)
    his = grid[-1].astype(np.float64) - taus  # clamp: min(x - tau, t11 - tau)
    return taus, his, rhos


def _rho128(rhos):
    """[128, 11] per-partition rho: partition p holds channel p % 32."""
    return np.ascontiguousarray(
        np.tile(np.asarray(rhos, dtype=np.float32), (J, 1))
    )  # [128, 11]


# --------------------------------------------------------------------------
# host-side layout marshalling (channel-on-partition)
# --------------------------------------------------------------------------
def shard_inputs(x):
    """Full x [B,H,W,C] -> per-core x0 arrays [T, 128, F], channel-on-partition.

    Per core: pixels n in [0, PIX), n = j*S + q, q = t*F + f.
    x0[t, j*32 + c, f] = shard[j*S + t*F + f, c].
    """
    xf = np.asarray(x, dtype=np.float32).reshape(N_CORES, PIX, C)
    out = []
    for i in range(N_CORES):
        a = xf[i].reshape(J, T, F, C).transpose(1, 0, 3, 2)  # [T, J, C, F]
        out.append(np.ascontiguousarray(a.reshape(T, P, F)))
    return out


def unshard_output(ys):
    """Inverse of shard_inputs for the per-core outputs y0 [T, 128, F]."""
    full = np.empty((N_CORES, PIX, C), dtype=np.float32)
    for i in range(N_CORES):
        a = np.asarray(ys[i]).reshape(T, J, C, F).transpose(1, 0, 3, 2)  # [J,T,F,C]
        full[i] = a.reshape(PIX, C)
    return full.reshape(B, H, WIDTH, C)


# --------------------------------------------------------------------------
# public entry
# --------------------------------------------------------------------------
def kernel(x: np.ndarray, grid: np.ndarray, W: np.ndarray) -> np.ndarray:
    from concourse.bass_utils import run_bass_kernel_spmd

    x = np.asarray(x)
    grid = np.asarray(grid)
    W = np.asarray(W)
    assert x.shape == (B, H, WIDTH, C) and grid.shape == (N_KNOTS,) and W.shape == (C, 8)

    key = (grid.tobytes(), W.tobytes())
    if _STATE.get("key") != key:
        taus, his, rhos = _constants(grid, W)
        _STATE["nc"] = build_module(taus, his, _rho128(rhos))
        _STATE["rho128"] = _rho128(rhos)
        _STATE["key"] = key
    nc = _STATE["nc"]
    rho128 = _STATE["rho128"]

    shards = shard_inputs(x)
    in_maps = [{"x0": shards[i], "rho0": rho128} for i in range(N_CORES)]
    res = run_bass_kernel_spmd(nc, in_maps, core_ids=list(range(N_CORES)))
    return unshard_output([r["y0"] for r in res.results])
